# revision 15
# baseline (speedup 1.0000x reference)
"""Trainium2 Bass kernel for nn_AsymmetricAttention (dense transformer block).

Strategy: tensor-parallel over heads across 8 NeuronCores (3 heads each).
Each core computes QKV for its heads, full attention over the 2240-token
concat stream, and partial output projections; the host sums the 8 partial
projections (the unshard step for the sum-sharded output).

All matmuls run in bf16 with fp32 PSUM accumulation. Layout convention on
device: "transposed" activations [feature, token] so every matmul contracts
on the partition axis without any on-device transposes:
  - q/k produced as q_T [hd, tok] (RoPE handled by permuting the head-dim
    order of the W rows to [even|odd] on the host),
  - v produced in natural [tok, hd] order by swapping stationary/moving
    operands,
  - attention computed as L^T = k_T.T @ q_T tiles; softmax denominator via
    a ones-row matmul; o_T = v.T @ p_T; projections consume o_T directly.
"""

import os
import numpy as np
import ml_dtypes

import concourse.bass as bass
import concourse.mybir as mybir
import concourse.tile as tile
from concourse import bacc
from concourse.bass_utils import run_bass_kernel_spmd

F32 = mybir.dt.float32
BF16 = mybir.dt.bfloat16

N_CORES = 8
H = 24
HC = H // N_CORES          # heads per core = 3
HD = 128
NX = 2048
DX = 3072
DY = 1536
NY = 256
EPS = 1e-6
INV_SQRT_HD = 1.0 / float(np.sqrt(HD))

KX = DX // 128             # 24 k-tiles over DX
KY = DY // 128             # 12 k-tiles over DY
NTX = NX // 128            # 16 token tiles for x
HALF = NX // 2             # token-half size (1024)

LAST_RESULT = None         # test harness reads exec_time_ns from here


def _ceil_div(a, b):
    return (a + b - 1) // b


def _chunks(total, step):
    out = []
    o = 0
    while o < total:
        out.append((o, min(step, total - o)))
        o += step
    return out


def build(crop, has_bqkv_x, has_bqkv_y):
    """Build the SPMD Bass graph (identical on all 8 cores)."""
    S = NX + crop
    kt_sizes = [(i * 128, min(128, S - i * 128)) for i in range(_ceil_div(S, 128))]
    qc_list = _chunks(S, 512)          # attention query chunks
    yt_sizes = [(o, n) for (o, n) in _chunks(crop, 128)]

    nc = bacc.Bacc("TRN2", target_bir_lowering=False, debug=False,
                   num_devices=N_CORES)

    # ---- DRAM I/O ----
    xT = nc.dram_tensor("xT", [DX, NX], F32, kind="ExternalInput")
    wqk_x = nc.dram_tensor("wqk_x", [DX, 6 * 128], F32, kind="ExternalInput")
    wv_x = nc.dram_tensor("wv_x", [DX, HC * 128], F32, kind="ExternalInput")
    sx_d = nc.dram_tensor("sx", [128, KX], F32, kind="ExternalInput")
    qn_d = nc.dram_tensor("qn", [128, 2 * HC], F32, kind="ExternalInput")  # x:q,k heads
    cos_d = nc.dram_tensor("cosT", [HC, 128, NX], F32, kind="ExternalInput")
    sin_d = nc.dram_tensor("sinT", [HC, 128, NX], F32, kind="ExternalInput")
    psw_d = nc.dram_tensor("psw", [128, 128], F32, kind="ExternalInput")
    bqk_x = nc.dram_tensor("bqk_x", [1, 6 * 128], F32, kind="ExternalInput")
    bv_x = nc.dram_tensor("bv_x", [1, HC * 128], F32, kind="ExternalInput")
    wpx_d = nc.dram_tensor("wpx", [HC * 128, DX], F32, kind="ExternalInput")
    px_d = nc.dram_tensor("px", [NX, DX], BF16, kind="ExternalOutput")
    if crop:
        yT = nc.dram_tensor("yT", [DY, crop], F32, kind="ExternalInput")
        wqk_y = nc.dram_tensor("wqk_y", [DY, 6 * 128], F32, kind="ExternalInput")
        wv_y = nc.dram_tensor("wv_y", [DY, HC * 128], F32, kind="ExternalInput")
        sy_d = nc.dram_tensor("sy", [128, KY], F32, kind="ExternalInput")
        qn_y_d = nc.dram_tensor("qn_y", [128, 2 * HC], F32, kind="ExternalInput")
        bqk_y = nc.dram_tensor("bqk_y", [1, 6 * 128], F32, kind="ExternalInput")
        bv_y = nc.dram_tensor("bv_y", [1, HC * 128], F32, kind="ExternalInput")
        wpy_d = nc.dram_tensor("wpy", [HC * 128, DY], F32, kind="ExternalInput")
        py_d = nc.dram_tensor("py", [crop, DY], BF16, kind="ExternalOutput")

    with tile.TileContext(nc) as tc:
        with tc.tile_pool(name="const", bufs=1) as constp, \
             tc.tile_pool(name="persist", bufs=1) as pers, \
             tc.tile_pool(name="ps", bufs=8, space="PSUM") as psp:

            # ---- constants ----
            ones_col = constp.tile([128, 1], BF16, tag="ones_col")
            nc.vector.memset(ones_col[:], 1.0)
            if has_bqkv_x or has_bqkv_y:
                ones_row = constp.tile([1, 512], BF16, tag="ones_row")
                nc.vector.memset(ones_row[:], 1.0)
            psw_sb = constp.tile([128, 128], BF16, tag="psw")
            nc.gpsimd.dma_start(psw_sb[:], psw_d[:])
            sx_sb = constp.tile([128, KX], F32, tag="sx")
            nc.sync.dma_start(sx_sb[:], sx_d[:])
            qn_sb = constp.tile([128, 2 * HC], F32, tag="qn")
            nc.sync.dma_start(qn_sb[:], qn_d[:])
            rxcol = constp.tile([128, NTX], F32, tag="rxcol")
            if crop:
                sy_sb = constp.tile([128, KY], F32, tag="sy")
                nc.sync.dma_start(sy_sb[:], sy_d[:])
                qny_sb = constp.tile([128, 2 * HC], F32, tag="qny")
                nc.sync.dma_start(qny_sb[:], qn_y_d[:])
                rycol = constp.tile([128, max(1, len(yt_sizes))], F32, tag="rycol")
            if has_bqkv_x:
                bqkx_sb = constp.tile([1, 6 * 128], BF16, tag="bqkx")
                nc.gpsimd.dma_start(bqkx_sb[:], bqk_x[:])
                bvx_sb = constp.tile([1, HC * 128], BF16, tag="bvx")
                nc.gpsimd.dma_start(bvx_sb[:], bv_x[:])
            if crop and has_bqkv_y:
                bqky_sb = constp.tile([1, 6 * 128], BF16, tag="bqky")
                nc.gpsimd.dma_start(bqky_sb[:], bqk_y[:])
                bvy_sb = constp.tile([1, HC * 128], BF16, tag="bvy")
                nc.gpsimd.dma_start(bvy_sb[:], bv_y[:])

            # ---- persistent activations ----
            q_T = [pers.tile([128, S], BF16, tag=f"qT{h}", name=f"qT{h}") for h in range(HC)]
            k_T = [pers.tile([128, S], BF16, tag=f"kT{h}", name=f"kT{h}") for h in range(HC)]
            v_all = pers.tile([128, len(kt_sizes), HC * 128], BF16, tag="v")

            # =========================================================
            # Phase A/B: x-stream QKV (two token halves)
            # =========================================================
            with tc.tile_pool(name="phA", bufs=1) as pA, \
                 tc.tile_pool(name="phAs", bufs=2) as pAs, \
                 tc.tile_pool(name="phAcs", bufs=1) as pAcs:

                # resident weights, cast to bf16 during DMA, then scaled by
                # (1 + scale_x) per input feature (per-partition scalar)
                wqk_sb = pA.tile([128, KX, 6 * 128], BF16, tag="wqk")
                nc.gpsimd.dma_start(
                    wqk_sb[:], wqk_x.ap().rearrange("(k p) f -> p k f", p=128))
                wv_sb = pA.tile([128, KX, HC * 128], BF16, tag="wv")
                nc.gpsimd.dma_start(
                    wv_sb[:], wv_x.ap().rearrange("(k p) f -> p k f", p=128))
                for kt in range(KX):
                    nc.vector.tensor_scalar(
                        out=wqk_sb[:, kt, :], in0=wqk_sb[:, kt, :],
                        scalar1=sx_sb[:, kt:kt + 1], scalar2=None,
                        op0=mybir.AluOpType.mult)
                    nc.vector.tensor_scalar(
                        out=wv_sb[:, kt, :], in0=wv_sb[:, kt, :],
                        scalar1=sx_sb[:, kt:kt + 1], scalar2=None,
                        op0=mybir.AluOpType.mult)

                for half in range(2):
                    t0 = half * HALF  # token offset
                    cs_tiles = {}
                    xg = []
                    for g in range(2):
                        xt = pAs.tile([128, KX // 2, HALF], BF16, tag="xh")
                        src = xT.ap()[g * (DX // 2):(g + 1) * (DX // 2),
                                      t0:t0 + HALF]
                        nc.gpsimd.dma_start(
                            xt[:], src.rearrange("(k p) t -> p k t", p=128))
                        xg.append(xt)

                    def xslice(kt, c0, cn):
                        t = xg[kt // (KX // 2)]
                        return t[:, kt % (KX // 2), c0:c0 + cn]

                    # r_x = rsqrt(mean(x^2)) for this half, via ones-matmul
                    rx_ps = [psp.tile([1, 512], F32, tag="bank", name="rx_ps") for _ in range(2)]
                    for kt in range(KX):
                        x2 = pAs.tile([128, HALF], BF16, tag="x2")
                        nc.vector.tensor_tensor(
                            out=x2[:], in0=xslice(kt, 0, HALF),
                            in1=xslice(kt, 0, HALF), op=mybir.AluOpType.mult)
                        for c in range(2):
                            nc.tensor.matmul(
                                rx_ps[c][:], ones_col[:], x2[:, c * 512:(c + 1) * 512],
                                start=(kt == 0), stop=(kt == KX - 1))
                    rx_row = pAs.tile([1, HALF], F32, tag="rxrow")
                    for c in range(2):
                        sl = rx_row[:, c * 512:(c + 1) * 512]
                        nc.vector.tensor_scalar(
                            out=sl, in0=rx_ps[c][:], scalar1=1.0 / DX, scalar2=EPS,
                            op0=mybir.AluOpType.mult, op1=mybir.AluOpType.add)
                        nc.vector.reciprocal(sl, sl)
                        nc.scalar.activation(sl, sl, mybir.ActivationFunctionType.Sqrt)
                    for t in range(8):
                        nc.sync.dma_start(
                            rxcol[:, half * 8 + t:half * 8 + t + 1],
                            rx_row[0:1, t * 128:(t + 1) * 128])

                    # q/k for the 6 feature tiles (q h0..2, k h0..2)
                    for m in range(6):
                        tens, h = divmod(m, HC)   # 0=q, 1=k
                        ps_qk = [psp.tile([128, 512], F32, tag="bank", name="ps_qk")
                                 for _ in range(2)]
                        if has_bqkv_x:
                            for c in range(2):
                                nc.tensor.matmul(
                                    ps_qk[c][:], bqkx_sb[:, m * 128:(m + 1) * 128],
                                    ones_row[:, 0:512], start=True, stop=False)
                        for kt in range(KX):
                            for c in range(2):
                                nc.tensor.matmul(
                                    ps_qk[c][:],
                                    wqk_sb[:, kt, m * 128:(m + 1) * 128],
                                    xslice(kt, c * 512, 512),
                                    start=(kt == 0 and not has_bqkv_x),
                                    stop=(kt == KX - 1))
                        # rope tables for this head+half (streamed once, used by q&k)
                        if tens == 0:
                            cs = pAcs.tile([128, HALF], BF16, tag=f"cos{h}")
                            sn = pAcs.tile([128, HALF], BF16, tag=f"sin{h}")
                            nc.gpsimd.dma_start(cs[:], cos_d.ap()[h, :, t0:t0 + HALF])
                            nc.gpsimd.dma_start(sn[:], sin_d.ap()[h, :, t0:t0 + HALF])
                            cs_tiles[h] = (cs, sn)
                        else:
                            cs, sn = cs_tiles[h]
                        for c in range(2):
                            qsb = pAs.tile([128, 512], BF16, tag="qsb")
                            nc.vector.tensor_copy(qsb[:], ps_qk[c][:])
                            x2q = pAs.tile([128, 512], BF16, tag="x2q")
                            nc.vector.tensor_tensor(
                                out=x2q[:], in0=qsb[:], in1=qsb[:],
                                op=mybir.AluOpType.mult)
                            ss = psp.tile([1, 512], F32, tag="bank")
                            nc.tensor.matmul(ss[:], ones_col[:], x2q[:],
                                             start=True, stop=True)
                            rr = pAs.tile([1, 512], F32, tag="rr")
                            nc.vector.tensor_scalar(
                                out=rr[:], in0=ss[:], scalar1=1.0 / HD, scalar2=EPS,
                                op0=mybir.AluOpType.mult, op1=mybir.AluOpType.add)
                            nc.vector.reciprocal(rr[:], rr[:])
                            nc.scalar.activation(
                                rr[:], rr[:], mybir.ActivationFunctionType.Sqrt)
                            rbc = pAs.tile([128, 512], F32, tag="rbc")
                            nc.gpsimd.partition_broadcast(rbc[:], rr[:])
                            qn_ap = qn_sb[:, m:m + 1]
                            qnorm = pAs.tile([128, 512], BF16, tag="qnorm")
                            nc.vector.scalar_tensor_tensor(
                                out=qnorm[:], in0=qsb[:], scalar=qn_ap, in1=rbc[:],
                                op0=mybir.AluOpType.mult, op1=mybir.AluOpType.mult)
                            # rope -> destination q_T/k_T columns.
                            # q' = q*cos2 + swap(q)*sin2, swap done on PE via
                            # a permutation matmul (DVE ops must be
                            # partition-aligned).
                            dst = (q_T if tens == 0 else k_T)[h]
                            dcol = slice(t0 + c * 512, t0 + (c + 1) * 512)
                            csl = cs[:, c * 512:(c + 1) * 512]
                            ssl = sn[:, c * 512:(c + 1) * 512]
                            qsw = psp.tile([128, 512], F32, tag="bank")
                            nc.tensor.matmul(qsw[:], psw_sb[:], qnorm[:],
                                             start=True, stop=True)
                            t1 = pAs.tile([128, 512], BF16, tag="ropet1")
                            nc.vector.tensor_tensor(out=t1[:], in0=qnorm[:],
                                                    in1=csl, op=mybir.AluOpType.mult)
                            t2 = pAs.tile([128, 512], BF16, tag="ropet2")
                            nc.vector.tensor_tensor(out=t2[:], in0=qsw[:], in1=ssl,
                                                    op=mybir.AluOpType.mult)
                            nc.vector.tensor_tensor(out=dst[:, dcol], in0=t1[:],
                                                    in1=t2[:], op=mybir.AluOpType.add)

                    # v for this half: natural [tok, hd] layout
                    for t in range(8):
                        gt = half * 8 + t
                        ps_v = psp.tile([128, HC * 128], F32, tag="bank")
                        if has_bqkv_x:
                            nc.tensor.matmul(
                                ps_v[:], ones_row[:, 0:128], bvx_sb[:],
                                start=True, stop=False)
                        for kt in range(KX):
                            nc.tensor.matmul(
                                ps_v[:], xslice(kt, t * 128, 128), wv_sb[:, kt, :],
                                start=(kt == 0 and not has_bqkv_x),
                                stop=(kt == KX - 1))
                        nc.scalar.activation(
                            v_all[:, gt, :], ps_v[:],
                            mybir.ActivationFunctionType.Copy,
                            scale=rxcol[:, gt:gt + 1])

            # =========================================================
            # Phase C: y-stream QKV
            # =========================================================
            if crop:
                with tc.tile_pool(name="phC", bufs=1) as pC, \
                     tc.tile_pool(name="phCs", bufs=2) as pCs:
                    y_bf = pC.tile([128, KY, crop], BF16, tag="ybf")
                    nc.gpsimd.dma_start(
                        y_bf[:], yT.ap().rearrange("(k p) t -> p k t", p=128))
                    wvy_sb = pC.tile([128, KY, HC * 128], BF16, tag="wvy")
                    nc.gpsimd.dma_start(
                        wvy_sb[:], wv_y.ap().rearrange("(k p) f -> p k f", p=128))
                    for kt in range(KY):
                        nc.vector.tensor_scalar(
                            out=wvy_sb[:, kt, :], in0=wvy_sb[:, kt, :],
                            scalar1=sy_sb[:, kt:kt + 1], scalar2=None,
                            op0=mybir.AluOpType.mult)

                    # r_y
                    ry_ps = psp.tile([1, 512], F32, tag="bank")
                    for kt in range(KY):
                        y2 = pCs.tile([128, crop], BF16, tag="y2")
                        nc.vector.tensor_tensor(
                            out=y2[:], in0=y_bf[:, kt, :], in1=y_bf[:, kt, :],
                            op=mybir.AluOpType.mult)
                        nc.tensor.matmul(ry_ps[0:1, 0:crop], ones_col[:], y2[:],
                                         start=(kt == 0), stop=(kt == KY - 1))
                    ry_row = pCs.tile([1, crop], F32, tag="ryrow")
                    nc.vector.tensor_scalar(
                        out=ry_row[:], in0=ry_ps[0:1, 0:crop], scalar1=1.0 / DY,
                        scalar2=EPS, op0=mybir.AluOpType.mult,
                        op1=mybir.AluOpType.add)
                    nc.vector.reciprocal(ry_row[:], ry_row[:])
                    nc.scalar.activation(ry_row[:], ry_row[:],
                                         mybir.ActivationFunctionType.Sqrt)
                    for ti, (o, n) in enumerate(yt_sizes):
                        nc.sync.dma_start(rycol[0:n, ti:ti + 1],
                                          ry_row[0:1, o:o + n])

                    # q_y / k_y
                    for m in range(6):
                        tens, h = divmod(m, HC)
                        wm = pCs.tile([128, KY, 128], BF16, tag="wqky")
                        nc.gpsimd.dma_start(
                            wm[:], wqk_y.ap()[:, m * 128:(m + 1) * 128]
                            .rearrange("(k p) f -> p k f", p=128))
                        for kt in range(KY):
                            nc.vector.tensor_scalar(
                                out=wm[:, kt, :], in0=wm[:, kt, :],
                                scalar1=sy_sb[:, kt:kt + 1], scalar2=None,
                                op0=mybir.AluOpType.mult)
                        ps_qy = psp.tile([128, 512], F32, tag="bank")
                        if has_bqkv_y:
                            nc.tensor.matmul(
                                ps_qy[:, 0:crop], bqky_sb[:, m * 128:(m + 1) * 128],
                                ones_row[:, 0:crop], start=True, stop=False)
                        for kt in range(KY):
                            nc.tensor.matmul(
                                ps_qy[:, 0:crop], wm[:, kt, :], y_bf[:, kt, :],
                                start=(kt == 0 and not has_bqkv_y),
                                stop=(kt == KY - 1))
                        qsb = pCs.tile([128, crop], BF16, tag="qysb")
                        nc.vector.tensor_copy(qsb[:], ps_qy[:, 0:crop])
                        x2q = pCs.tile([128, crop], BF16, tag="x2qy")
                        nc.vector.tensor_tensor(out=x2q[:], in0=qsb[:], in1=qsb[:],
                                                op=mybir.AluOpType.mult)
                        ss = psp.tile([1, 512], F32, tag="bank")
                        nc.tensor.matmul(ss[0:1, 0:crop], ones_col[:], x2q[:],
                                         start=True, stop=True)
                        rr = pCs.tile([1, crop], F32, tag="rry")
                        nc.vector.tensor_scalar(
                            out=rr[:], in0=ss[0:1, 0:crop], scalar1=1.0 / HD,
                            scalar2=EPS, op0=mybir.AluOpType.mult,
                            op1=mybir.AluOpType.add)
                        nc.vector.reciprocal(rr[:], rr[:])
                        nc.scalar.activation(rr[:], rr[:],
                                             mybir.ActivationFunctionType.Sqrt)
                        rbc = pCs.tile([128, crop], F32, tag="rbcy")
                        nc.gpsimd.partition_broadcast(rbc[:], rr[:])
                        dst = (q_T if tens == 0 else k_T)[h]
                        nc.vector.scalar_tensor_tensor(
                            out=dst[:, NX:NX + crop], in0=qsb[:],
                            scalar=qny_sb[:, m:m + 1], in1=rbc[:],
                            op0=mybir.AluOpType.mult, op1=mybir.AluOpType.mult)

                    # v_y
                    for ti, (o, n) in enumerate(yt_sizes):
                        gt = NTX + ti
                        ps_vy = psp.tile([128, HC * 128], F32, tag="bank")
                        if has_bqkv_y:
                            nc.tensor.matmul(
                                ps_vy[0:n, :], ones_row[:, 0:n], bvy_sb[:],
                                start=True, stop=False)
                        for kt in range(KY):
                            nc.tensor.matmul(
                                ps_vy[0:n, :], y_bf[:, kt, o:o + n], wvy_sb[:, kt, :],
                                start=(kt == 0 and not has_bqkv_y),
                                stop=(kt == KY - 1))
                        nc.scalar.activation(
                            v_all[0:n, gt, :], ps_vy[0:n, :],
                            mybir.ActivationFunctionType.Copy,
                            scale=rycol[0:n, ti:ti + 1])

            # =========================================================
            # Phase D: attention + projections, pipelined per query chunk
            # =========================================================
            with tc.tile_pool(name="phD", bufs=1) as pD, \
                 tc.tile_pool(name="phDs", bufs=4) as pDs, \
                 tc.tile_pool(name="phDo", bufs=2) as pDo:
                wpx_sb = pD.tile([128, HC, DX], BF16, tag="wpx")
                nc.gpsimd.dma_start(
                    wpx_sb[:], wpx_d.ap().rearrange("(h p) f -> p h f", p=128))
                if crop:
                    wpy_sb = pD.tile([128, HC, DY], BF16, tag="wpy")
                    nc.gpsimd.dma_start(
                        wpy_sb[:], wpy_d.ap().rearrange("(h p) f -> p h f", p=128))

                for qi, (q0, qn) in enumerate(qc_list):
                    o_sb = []
                    for h in range(HC):
                        o_ps = psp.tile([128, 512], F32, tag="bank")
                        d_ps = psp.tile([1, 512], F32, tag="bank")
                        nkt = len(kt_sizes)
                        for ki, (k0, kn) in enumerate(kt_sizes):
                            l_ps = psp.tile([128, 512], F32, tag="bank")
                            nc.tensor.matmul(
                                l_ps[0:kn, 0:qn], k_T[h][:, k0:k0 + kn],
                                q_T[h][:, q0:q0 + qn], start=True, stop=True)
                            p_sb = pDs.tile([128, 512], BF16, tag="pT")
                            nc.scalar.activation(
                                p_sb[0:kn, 0:qn], l_ps[0:kn, 0:qn],
                                mybir.ActivationFunctionType.Exp,
                                scale=INV_SQRT_HD)
                            nc.tensor.matmul(
                                o_ps[:, 0:qn], v_all[0:kn, ki, h * 128:(h + 1) * 128],
                                p_sb[0:kn, 0:qn], start=(ki == 0),
                                stop=(ki == nkt - 1))
                            nc.tensor.matmul(
                                d_ps[0:1, 0:qn], ones_col[0:kn, :],
                                p_sb[0:kn, 0:qn], start=(ki == 0),
                                stop=(ki == nkt - 1))
                        rd = pDs.tile([1, 512], F32, tag="rd")
                        nc.vector.reciprocal(rd[0:1, 0:qn], d_ps[0:1, 0:qn])
                        rdb = pDs.tile([128, 512], F32, tag="rdb")
                        nc.gpsimd.partition_broadcast(rdb[:, 0:qn], rd[0:1, 0:qn])
                        ot = pDs.tile([128, 512], BF16, tag=f"oT{h}")
                        nc.vector.tensor_tensor(
                            out=ot[:, 0:qn], in0=o_ps[:, 0:qn], in1=rdb[:, 0:qn],
                            op=mybir.AluOpType.mult)
                        o_sb.append(ot)

                    # projections for this query chunk
                    in_x = q0 < NX
                    if in_x:
                        D, wp, out_d = DX, wpx_sb, px_d
                    else:
                        D, wp, out_d = DY, wpy_sb, py_d
                    nf_chunks = _chunks(D, 512)
                    for (p0, pn) in _chunks(qn, 128):
                        stage = pDo.tile([128, D], BF16, tag=f"stage{int(in_x)}")
                        for gi in range(0, len(nf_chunks), 3):
                            grp = nf_chunks[gi:gi + 3]
                            ps_p = [psp.tile([128, 512], F32, tag="bank", name="ps_p")
                                    for _ in grp]
                            for h in range(HC):
                                for j, (f0, fn) in enumerate(grp):
                                    nc.tensor.matmul(
                                        ps_p[j][0:pn, 0:fn],
                                        o_sb[h][:, p0:p0 + pn],
                                        wp[:, h, f0:f0 + fn],
                                        start=(h == 0), stop=(h == HC - 1))
                            for j, (f0, fn) in enumerate(grp):
                                eng = nc.vector if (gi + j) % 2 == 0 else nc.scalar
                                if eng is nc.vector:
                                    nc.vector.tensor_copy(
                                        stage[0:pn, f0:f0 + fn], ps_p[j][0:pn, 0:fn])
                                else:
                                    nc.scalar.copy(
                                        stage[0:pn, f0:f0 + fn], ps_p[j][0:pn, 0:fn])
                        r0 = q0 + p0 - (0 if in_x else NX)
                        nc.sync.dma_start(out_d.ap()[r0:r0 + pn, :], stage[0:pn, :])

    nc.compile()
    return nc


_CACHE = {}


def _get_nc(crop, has_bqkv_x, has_bqkv_y):
    key = (crop, has_bqkv_x, has_bqkv_y)
    if key not in _CACHE:
        _CACHE[key] = build(*key)
    return _CACHE[key]


def _install_profile_hook():
    try:
        from antenv.axon_hooks import (get_axon_ntff_profile_hook,
                                       set_axon_ntff_profile_hook)
        if get_axon_ntff_profile_hook() is None:
            from trn_agent_boot.trn_boot import _ntff_profile_via_ctypes
            set_axon_ntff_profile_hook(
                _ntff_profile_via_ctypes("/opt/axon/libaxon_pjrt.so"))
    except Exception:
        pass


# permutation putting even head-dims first, odd second (for on-chip RoPE)
_PERM = np.concatenate([np.arange(0, HD, 2), np.arange(1, HD, 2)])
# partition-swap matrix (p <-> p+64), symmetric involution
_PSW = np.zeros((HD, HD), np.float32)
_PSW[np.arange(HD), (np.arange(HD) + 64) % HD] = 1.0


def kernel(x, y, scale_x, scale_y, rope_cos, rope_sin,
           w_qkv_x, b_qkv_x, w_qkv_y, b_qkv_y,
           qnx_w, knx_w, qny_w, kny_w,
           w_proj_x, b_proj_x, w_proj_y, b_proj_y, crop_y):
    global LAST_RESULT
    crop = int(crop_y)
    f32 = np.float32
    x = np.asarray(x, f32)
    y = np.asarray(y, f32)

    has_bx = bool(np.any(np.asarray(b_qkv_x)))
    has_by = bool(np.any(np.asarray(b_qkv_y)))
    trace = bool(os.environ.get("BASS_TRACE"))
    if trace:
        _install_profile_hook()
    nc = _get_nc(crop, has_bx, has_by)

    xT = np.ascontiguousarray(x[0].T)
    sx = (1.0 + np.asarray(scale_x, f32)[0])
    sy = (1.0 + np.asarray(scale_y, f32)[0])
    wqx = np.asarray(w_qkv_x, f32).reshape(3, H, HD, DX)
    wqy = np.asarray(w_qkv_y, f32).reshape(3, H, HD, DY)
    bqx = np.asarray(b_qkv_x, f32).reshape(3, H, HD)
    bqy = np.asarray(b_qkv_y, f32).reshape(3, H, HD)
    wpx = np.asarray(w_proj_x, f32)
    wpy = np.asarray(w_proj_y, f32)
    cos = np.asarray(rope_cos, f32)
    sin = np.asarray(rope_sin, f32)
    qn = {0: np.asarray(qnx_w, f32), 1: np.asarray(knx_w, f32)}
    qny = {0: np.asarray(qny_w, f32), 1: np.asarray(kny_w, f32)}

    in_maps = []
    for c in range(N_CORES):
        hs = slice(c * HC, (c + 1) * HC)
        m = {}
        m["xT"] = xT
        # q/k weight block: [q h0..2 | k h0..2], head-dim rows permuted
        wqk = np.concatenate([wqx[t, hs][:, _PERM, :].reshape(HC * HD, DX)
                              for t in (0, 1)], axis=0)
        m["wqk_x"] = np.ascontiguousarray(wqk.T)
        m["wv_x"] = np.ascontiguousarray(wqx[2, hs].reshape(HC * HD, DX).T)
        m["sx"] = np.ascontiguousarray(sx.reshape(KX, 128).T)
        m["qn"] = np.ascontiguousarray(
            np.stack([qn[0][_PERM]] * HC + [qn[1][_PERM]] * HC, axis=1))
        # rope tables in [head, hd-partition, token] layout:
        #   cos2[p] = cos[p%64];  sin2[p<64] = -sin[p], sin2[p>=64] = +sin[p-64]
        cosh = cos[:, hs, :].transpose(1, 2, 0)        # (HC, 64, NX)
        sinh = sin[:, hs, :].transpose(1, 2, 0)
        m["cosT"] = np.ascontiguousarray(
            np.concatenate([cosh, cosh], axis=1))
        m["sinT"] = np.ascontiguousarray(
            np.concatenate([-sinh, sinh], axis=1))
        m["psw"] = _PSW
        m["bqk_x"] = np.concatenate(
            [bqx[t, hs][:, _PERM].reshape(1, HC * HD) for t in (0, 1)], axis=1)
        m["bv_x"] = bqx[2, hs].reshape(1, HC * HD)
        m["wpx"] = np.ascontiguousarray(wpx[:, c * HC * HD:(c + 1) * HC * HD].T)
        if crop:
            m["yT"] = np.ascontiguousarray(y[0, :crop].T)
            wqky = np.concatenate([wqy[t, hs][:, _PERM, :].reshape(HC * HD, DY)
                                   for t in (0, 1)], axis=0)
            m["wqk_y"] = np.ascontiguousarray(wqky.T)
            m["wv_y"] = np.ascontiguousarray(wqy[2, hs].reshape(HC * HD, DY).T)
            m["sy"] = np.ascontiguousarray(sy.reshape(KY, 128).T)
            m["qn_y"] = np.ascontiguousarray(
                np.stack([qny[0][_PERM]] * HC + [qny[1][_PERM]] * HC, axis=1))
            m["bqk_y"] = np.concatenate(
                [bqy[t, hs][:, _PERM].reshape(1, HC * HD) for t in (0, 1)], axis=1)
            m["bv_y"] = bqy[2, hs].reshape(1, HC * HD)
            m["wpy"] = np.ascontiguousarray(wpy[:, c * HC * HD:(c + 1) * HC * HD].T)
        in_maps.append({k: np.ascontiguousarray(v, f32) for k, v in m.items()})

    res = run_bass_kernel_spmd(nc, in_maps, core_ids=list(range(N_CORES)),
                               trace=trace)
    LAST_RESULT = res

    x_out = np.zeros((NX, DX), f32)
    y_out = np.zeros((NY, DY), f32)
    for c in range(N_CORES):
        x_out += res.results[c]["px"].astype(f32)
        if crop:
            y_out[:crop] += res.results[c]["py"].astype(f32)
    x_out += np.asarray(b_proj_x, f32)[None, :]
    y_out += np.asarray(b_proj_y, f32)[None, :]
    return x_out[None], y_out[None]


# revision 19
# speedup vs baseline: 1.3915x; 1.3915x over previous
"""Trainium2 Bass kernel for nn_AsymmetricAttention (dense transformer block).

Strategy: tensor-parallel over heads across 8 NeuronCores (3 heads each).
Each core computes QKV for its heads, full attention over the 2240-token
concat stream, and partial output projections; the host sums the 8 partial
projections (the unshard step for the sum-sharded output).

All matmuls run in bf16 with fp32 PSUM accumulation. Layout convention on
device: "transposed" activations [feature, token] so every matmul contracts
on the partition axis without any on-device transposes:
  - q/k produced as q_T [hd, tok] (RoPE handled by permuting the head-dim
    order of the W rows to [even|odd] on the host),
  - v produced in natural [tok, hd] order by swapping stationary/moving
    operands,
  - attention computed as L^T = k_T.T @ q_T tiles; softmax denominator via
    a ones-row matmul; o_T = v.T @ p_T; projections consume o_T directly.
"""

import os
import numpy as np
import ml_dtypes

import concourse.bass as bass
import concourse.mybir as mybir
import concourse.tile as tile
from concourse import bacc
from concourse.bass_utils import run_bass_kernel_spmd

F32 = mybir.dt.float32
BF16 = mybir.dt.bfloat16

N_CORES = 8
H = 24
HC = H // N_CORES          # heads per core = 3
HD = 128
NX = 2048
DX = 3072
DY = 1536
NY = 256
EPS = 1e-6
INV_SQRT_HD = 1.0 / float(np.sqrt(HD))

KX = DX // 128             # 24 k-tiles over DX
KY = DY // 128             # 12 k-tiles over DY
NTX = NX // 128            # 16 token tiles for x
HALF = NX // 2             # token-half size (1024)

LAST_RESULT = None         # test harness reads exec_time_ns from here


def _ceil_div(a, b):
    return (a + b - 1) // b


def _chunks(total, step):
    out = []
    o = 0
    while o < total:
        out.append((o, min(step, total - o)))
        o += step
    return out


def build(crop, has_bqkv_x, has_bqkv_y):
    """Build the SPMD Bass graph (identical on all 8 cores)."""
    S = NX + crop
    kt_sizes = [(i * 128, min(128, S - i * 128)) for i in range(_ceil_div(S, 128))]
    qc_list = _chunks(S, 512)          # attention query chunks
    yt_sizes = [(o, n) for (o, n) in _chunks(crop, 128)]

    nc = bacc.Bacc("TRN2", target_bir_lowering=False, debug=False,
                   num_devices=N_CORES)

    # ---- DRAM I/O ----
    xT = nc.dram_tensor("xT", [DX, NX], F32, kind="ExternalInput")
    wqk_x = nc.dram_tensor("wqk_x", [DX, 6 * 128], F32, kind="ExternalInput")
    wv_x = nc.dram_tensor("wv_x", [DX, HC * 128], F32, kind="ExternalInput")
    sx_d = nc.dram_tensor("sx", [128, KX], F32, kind="ExternalInput")
    qn_d = nc.dram_tensor("qn", [128, 2 * HC], F32, kind="ExternalInput")  # x:q,k heads
    cos_d = nc.dram_tensor("cosT", [HC, 128, NX], F32, kind="ExternalInput")
    sin_d = nc.dram_tensor("sinT", [HC, 128, NX], F32, kind="ExternalInput")
    psw_d = nc.dram_tensor("psw", [128, 128], F32, kind="ExternalInput")
    bqk_x = nc.dram_tensor("bqk_x", [1, 6 * 128], F32, kind="ExternalInput")
    bv_x = nc.dram_tensor("bv_x", [1, HC * 128], F32, kind="ExternalInput")
    wpx_d = nc.dram_tensor("wpx", [HC * 128, DX], F32, kind="ExternalInput")
    px_d = nc.dram_tensor("px", [NX, DX], BF16, kind="ExternalOutput")
    if crop:
        yT = nc.dram_tensor("yT", [DY, crop], F32, kind="ExternalInput")
        wqk_y = nc.dram_tensor("wqk_y", [DY, 6 * 128], F32, kind="ExternalInput")
        wv_y = nc.dram_tensor("wv_y", [DY, HC * 128], F32, kind="ExternalInput")
        sy_d = nc.dram_tensor("sy", [128, KY], F32, kind="ExternalInput")
        qn_y_d = nc.dram_tensor("qn_y", [128, 2 * HC], F32, kind="ExternalInput")
        bqk_y = nc.dram_tensor("bqk_y", [1, 6 * 128], F32, kind="ExternalInput")
        bv_y = nc.dram_tensor("bv_y", [1, HC * 128], F32, kind="ExternalInput")
        wpy_d = nc.dram_tensor("wpy", [HC * 128, DY], F32, kind="ExternalInput")
        py_d = nc.dram_tensor("py", [crop, DY], BF16, kind="ExternalOutput")

    with tile.TileContext(nc) as tc:
        with tc.tile_pool(name="const", bufs=1) as constp, \
             tc.tile_pool(name="persist", bufs=1) as pers, \
             tc.tile_pool(name="ps", bufs=8, space="PSUM") as psp:

            # ---- constants ----
            ones_col = constp.tile([128, 1], BF16, tag="ones_col")
            nc.vector.memset(ones_col[:], 1.0)
            if has_bqkv_x or has_bqkv_y:
                ones_row = constp.tile([1, 512], BF16, tag="ones_row")
                nc.vector.memset(ones_row[:], 1.0)
            psw_sb = constp.tile([128, 128], BF16, tag="psw")
            nc.gpsimd.dma_start(psw_sb[:], psw_d[:])
            sx_sb = constp.tile([128, KX], F32, tag="sx")
            nc.sync.dma_start(sx_sb[:], sx_d[:])
            qn_sb = constp.tile([128, 2 * HC], F32, tag="qn")
            nc.sync.dma_start(qn_sb[:], qn_d[:])
            rxcol = constp.tile([128, NTX], F32, tag="rxcol")
            if crop:
                sy_sb = constp.tile([128, KY], F32, tag="sy")
                nc.sync.dma_start(sy_sb[:], sy_d[:])
                qny_sb = constp.tile([128, 2 * HC], F32, tag="qny")
                nc.sync.dma_start(qny_sb[:], qn_y_d[:])
                rycol = constp.tile([128, max(1, len(yt_sizes))], F32, tag="rycol")
            if has_bqkv_x:
                bqkx_sb = constp.tile([1, 6 * 128], BF16, tag="bqkx")
                nc.gpsimd.dma_start(bqkx_sb[:], bqk_x[:])
                bvx_sb = constp.tile([1, HC * 128], BF16, tag="bvx")
                nc.gpsimd.dma_start(bvx_sb[:], bv_x[:])
            if crop and has_bqkv_y:
                bqky_sb = constp.tile([1, 6 * 128], BF16, tag="bqky")
                nc.gpsimd.dma_start(bqky_sb[:], bqk_y[:])
                bvy_sb = constp.tile([1, HC * 128], BF16, tag="bvy")
                nc.gpsimd.dma_start(bvy_sb[:], bv_y[:])

            # ---- persistent activations ----
            q_T = [pers.tile([128, S], BF16, tag=f"qT{h}", name=f"qT{h}") for h in range(HC)]
            k_T = [pers.tile([128, S], BF16, tag=f"kT{h}", name=f"kT{h}") for h in range(HC)]
            v_all = pers.tile([128, len(kt_sizes), HC * 128], BF16, tag="v")

            # =========================================================
            # Phase A/B: x-stream QKV (two token halves)
            # =========================================================
            with tc.tile_pool(name="phA", bufs=1) as pA, \
                 tc.tile_pool(name="phAs", bufs=2) as pAs, \
                 tc.tile_pool(name="phAcs", bufs=1) as pAcs:

                # resident weights, cast to bf16 during DMA, then scaled by
                # (1 + scale_x) per input feature (per-partition scalar)
                wqk_sb = pA.tile([128, KX, 6 * 128], BF16, tag="wqk")
                nc.gpsimd.dma_start(
                    wqk_sb[:], wqk_x.ap().rearrange("(k p) f -> p k f", p=128))
                wv_sb = pA.tile([128, KX, HC * 128], BF16, tag="wv")
                nc.gpsimd.dma_start(
                    wv_sb[:], wv_x.ap().rearrange("(k p) f -> p k f", p=128))
                for kt in range(KX):
                    nc.vector.tensor_scalar(
                        out=wqk_sb[:, kt, :], in0=wqk_sb[:, kt, :],
                        scalar1=sx_sb[:, kt:kt + 1], scalar2=None,
                        op0=mybir.AluOpType.mult)
                    nc.vector.tensor_scalar(
                        out=wv_sb[:, kt, :], in0=wv_sb[:, kt, :],
                        scalar1=sx_sb[:, kt:kt + 1], scalar2=None,
                        op0=mybir.AluOpType.mult)

                for half in range(2):
                    t0 = half * HALF  # token offset
                    cs_tiles = {}
                    xg = []
                    for g in range(2):
                        xt = pAs.tile([128, KX // 2, HALF], BF16, tag="xh")
                        src = xT.ap()[g * (DX // 2):(g + 1) * (DX // 2),
                                      t0:t0 + HALF]
                        nc.gpsimd.dma_start(
                            xt[:], src.rearrange("(k p) t -> p k t", p=128))
                        xg.append(xt)

                    def xslice(kt, c0, cn):
                        t = xg[kt // (KX // 2)]
                        return t[:, kt % (KX // 2), c0:c0 + cn]

                    # r_x = rsqrt(mean(x^2)) for this half, via ones-matmul
                    rx_ps = [psp.tile([1, 512], F32, tag="bank", name="rx_ps") for _ in range(2)]
                    for kt in range(KX):
                        x2 = pAs.tile([128, HALF], BF16, tag="x2")
                        nc.vector.tensor_tensor(
                            out=x2[:], in0=xslice(kt, 0, HALF),
                            in1=xslice(kt, 0, HALF), op=mybir.AluOpType.mult)
                        for c in range(2):
                            nc.tensor.matmul(
                                rx_ps[c][:], ones_col[:], x2[:, c * 512:(c + 1) * 512],
                                start=(kt == 0), stop=(kt == KX - 1))
                    rx_row = pAs.tile([1, HALF], F32, tag="rxrow")
                    for c in range(2):
                        sl = rx_row[:, c * 512:(c + 1) * 512]
                        nc.vector.tensor_scalar(
                            out=sl, in0=rx_ps[c][:], scalar1=1.0 / DX, scalar2=EPS,
                            op0=mybir.AluOpType.mult, op1=mybir.AluOpType.add)
                        nc.vector.reciprocal_approx_fast(sl, sl)
                        nc.scalar.activation(sl, sl, mybir.ActivationFunctionType.Sqrt)
                    for t in range(8):
                        nc.sync.dma_start(
                            rxcol[:, half * 8 + t:half * 8 + t + 1],
                            rx_row[0:1, t * 128:(t + 1) * 128])

                    # q/k for the 6 feature tiles (q h0..2, k h0..2).
                    # Software-pipelined: the m-th epilogue (whose ss / rope
                    # matmuls depend on a long DVE chain) is emitted after the
                    # (m+1)-th accumulation matmuls so PE never stalls.
                    def qk_mms(m):
                        tens, h = divmod(m, HC)   # 0=q, 1=k
                        ps_qk = [psp.tile([128, 512], F32, tag="bank",
                                          name="ps_qk") for _ in range(2)]
                        if has_bqkv_x:
                            for c in range(2):
                                nc.tensor.matmul(
                                    ps_qk[c][:], bqkx_sb[:, m * 128:(m + 1) * 128],
                                    ones_row[:, 0:512], start=True, stop=False)
                        for kt in range(KX):
                            for c in range(2):
                                nc.tensor.matmul(
                                    ps_qk[c][:],
                                    wqk_sb[:, kt, m * 128:(m + 1) * 128],
                                    xslice(kt, c * 512, 512),
                                    start=(kt == 0 and not has_bqkv_x),
                                    stop=(kt == KX - 1))
                        # rope tables for this head+half (streamed at q, reused at k)
                        if tens == 0:
                            cs = pAcs.tile([128, HALF], BF16, tag=f"cos{h}")
                            sn = pAcs.tile([128, HALF], BF16, tag=f"sin{h}")
                            nc.gpsimd.dma_start(cs[:], cos_d.ap()[h, :, t0:t0 + HALF])
                            nc.gpsimd.dma_start(sn[:], sin_d.ap()[h, :, t0:t0 + HALF])
                            cs_tiles[h] = (cs, sn)
                        return ps_qk

                    def qk_epilogue(m, ps_qk):
                        tens, h = divmod(m, HC)
                        cs, sn = cs_tiles[h]
                        for c in range(2):
                            qsb = pAs.tile([128, 512], BF16, tag="qsb")
                            nc.vector.tensor_copy(qsb[:], ps_qk[c][:])
                            x2q = pAs.tile([128, 512], BF16, tag="x2q")
                            nc.vector.tensor_tensor(
                                out=x2q[:], in0=qsb[:], in1=qsb[:],
                                op=mybir.AluOpType.mult)
                            ss = psp.tile([1, 512], F32, tag="bank")
                            nc.tensor.matmul(ss[:], ones_col[:], x2q[:],
                                             start=True, stop=True)
                            rr = pAs.tile([1, 512], F32, tag="rr")
                            nc.vector.tensor_scalar(
                                out=rr[:], in0=ss[:], scalar1=1.0 / HD, scalar2=EPS,
                                op0=mybir.AluOpType.mult, op1=mybir.AluOpType.add)
                            nc.vector.reciprocal_approx_fast(rr[:], rr[:])
                            nc.scalar.activation(
                                rr[:], rr[:], mybir.ActivationFunctionType.Sqrt)
                            rbc = pAs.tile([128, 512], F32, tag="rbc")
                            nc.gpsimd.partition_broadcast(rbc[:], rr[:])
                            qn_ap = qn_sb[:, m:m + 1]
                            qnorm = pAs.tile([128, 512], BF16, tag="qnorm")
                            nc.vector.scalar_tensor_tensor(
                                out=qnorm[:], in0=qsb[:], scalar=qn_ap, in1=rbc[:],
                                op0=mybir.AluOpType.mult, op1=mybir.AluOpType.mult)
                            # rope: q' = q*cos2 + swap(q)*sin2 (swap on PE via a
                            # permutation matmul; DVE ops must be partition-aligned)
                            dst = (q_T if tens == 0 else k_T)[h]
                            dcol = slice(t0 + c * 512, t0 + (c + 1) * 512)
                            csl = cs[:, c * 512:(c + 1) * 512]
                            ssl = sn[:, c * 512:(c + 1) * 512]
                            qsw = psp.tile([128, 512], F32, tag="bank")
                            nc.tensor.matmul(qsw[:], psw_sb[:], qnorm[:],
                                             start=True, stop=True)
                            t1 = pAs.tile([128, 512], BF16, tag="ropet1")
                            nc.vector.tensor_tensor(out=t1[:], in0=qnorm[:],
                                                    in1=csl, op=mybir.AluOpType.mult)
                            t2 = pAs.tile([128, 512], BF16, tag="ropet2")
                            nc.vector.tensor_tensor(out=t2[:], in0=qsw[:], in1=ssl,
                                                    op=mybir.AluOpType.mult)
                            nc.vector.tensor_tensor(out=dst[:, dcol], in0=t1[:],
                                                    in1=t2[:], op=mybir.AluOpType.add)

                    pend = None
                    for m in range(6):
                        ps_qk = qk_mms(m)
                        if pend is not None:
                            qk_epilogue(*pend)
                        pend = (m, ps_qk)

                    # v for this half: natural [tok, hd] layout
                    for t in range(8):
                        gt = half * 8 + t
                        ps_v = psp.tile([128, HC * 128], F32, tag="bank")
                        if has_bqkv_x:
                            nc.tensor.matmul(
                                ps_v[:], ones_row[:, 0:128], bvx_sb[:],
                                start=True, stop=False)
                        for kt in range(KX):
                            nc.tensor.matmul(
                                ps_v[:], xslice(kt, t * 128, 128), wv_sb[:, kt, :],
                                start=(kt == 0 and not has_bqkv_x),
                                stop=(kt == KX - 1))
                        if pend is not None:
                            qk_epilogue(*pend)
                            pend = None
                        nc.scalar.activation(
                            v_all[:, gt, :], ps_v[:],
                            mybir.ActivationFunctionType.Copy,
                            scale=rxcol[:, gt:gt + 1])

            # =========================================================
            # Phase C: y-stream QKV
            # =========================================================
            if crop:
                with tc.tile_pool(name="phC", bufs=1) as pC, \
                     tc.tile_pool(name="phCs", bufs=2) as pCs:
                    y_bf = pC.tile([128, KY, crop], BF16, tag="ybf")
                    nc.gpsimd.dma_start(
                        y_bf[:], yT.ap().rearrange("(k p) t -> p k t", p=128))
                    wvy_sb = pC.tile([128, KY, HC * 128], BF16, tag="wvy")
                    nc.gpsimd.dma_start(
                        wvy_sb[:], wv_y.ap().rearrange("(k p) f -> p k f", p=128))
                    for kt in range(KY):
                        nc.vector.tensor_scalar(
                            out=wvy_sb[:, kt, :], in0=wvy_sb[:, kt, :],
                            scalar1=sy_sb[:, kt:kt + 1], scalar2=None,
                            op0=mybir.AluOpType.mult)

                    # r_y
                    ry_ps = psp.tile([1, 512], F32, tag="bank")
                    for kt in range(KY):
                        y2 = pCs.tile([128, crop], BF16, tag="y2")
                        nc.vector.tensor_tensor(
                            out=y2[:], in0=y_bf[:, kt, :], in1=y_bf[:, kt, :],
                            op=mybir.AluOpType.mult)
                        nc.tensor.matmul(ry_ps[0:1, 0:crop], ones_col[:], y2[:],
                                         start=(kt == 0), stop=(kt == KY - 1))
                    ry_row = pCs.tile([1, crop], F32, tag="ryrow")
                    nc.vector.tensor_scalar(
                        out=ry_row[:], in0=ry_ps[0:1, 0:crop], scalar1=1.0 / DY,
                        scalar2=EPS, op0=mybir.AluOpType.mult,
                        op1=mybir.AluOpType.add)
                    nc.vector.reciprocal_approx_fast(ry_row[:], ry_row[:])
                    nc.scalar.activation(ry_row[:], ry_row[:],
                                         mybir.ActivationFunctionType.Sqrt)
                    for ti, (o, n) in enumerate(yt_sizes):
                        nc.sync.dma_start(rycol[0:n, ti:ti + 1],
                                          ry_row[0:1, o:o + n])

                    # q_y / k_y (1-deep software pipeline, like the x stream)
                    def qky_mms(m):
                        wm = pCs.tile([128, KY, 128], BF16, tag="wqky",
                                      name="wqky")
                        nc.gpsimd.dma_start(
                            wm[:], wqk_y.ap()[:, m * 128:(m + 1) * 128]
                            .rearrange("(k p) f -> p k f", p=128))
                        for kt in range(KY):
                            nc.vector.tensor_scalar(
                                out=wm[:, kt, :], in0=wm[:, kt, :],
                                scalar1=sy_sb[:, kt:kt + 1], scalar2=None,
                                op0=mybir.AluOpType.mult)
                        ps_qy = psp.tile([128, 512], F32, tag="bank",
                                         name="ps_qy")
                        if has_bqkv_y:
                            nc.tensor.matmul(
                                ps_qy[:, 0:crop], bqky_sb[:, m * 128:(m + 1) * 128],
                                ones_row[:, 0:crop], start=True, stop=False)
                        for kt in range(KY):
                            nc.tensor.matmul(
                                ps_qy[:, 0:crop], wm[:, kt, :], y_bf[:, kt, :],
                                start=(kt == 0 and not has_bqkv_y),
                                stop=(kt == KY - 1))
                        return ps_qy

                    def qky_epilogue(m, ps_qy):
                        tens, h = divmod(m, HC)
                        qsb = pCs.tile([128, crop], BF16, tag="qysb")
                        nc.vector.tensor_copy(qsb[:], ps_qy[:, 0:crop])
                        x2q = pCs.tile([128, crop], BF16, tag="x2qy")
                        nc.vector.tensor_tensor(out=x2q[:], in0=qsb[:], in1=qsb[:],
                                                op=mybir.AluOpType.mult)
                        ss = psp.tile([1, 512], F32, tag="bank")
                        nc.tensor.matmul(ss[0:1, 0:crop], ones_col[:], x2q[:],
                                         start=True, stop=True)
                        rr = pCs.tile([1, crop], F32, tag="rry")
                        nc.vector.tensor_scalar(
                            out=rr[:], in0=ss[0:1, 0:crop], scalar1=1.0 / HD,
                            scalar2=EPS, op0=mybir.AluOpType.mult,
                            op1=mybir.AluOpType.add)
                        nc.vector.reciprocal_approx_fast(rr[:], rr[:])
                        nc.scalar.activation(rr[:], rr[:],
                                             mybir.ActivationFunctionType.Sqrt)
                        rbc = pCs.tile([128, crop], F32, tag="rbcy")
                        nc.gpsimd.partition_broadcast(rbc[:], rr[:])
                        dst = (q_T if tens == 0 else k_T)[h]
                        nc.vector.scalar_tensor_tensor(
                            out=dst[:, NX:NX + crop], in0=qsb[:],
                            scalar=qny_sb[:, m:m + 1], in1=rbc[:],
                            op0=mybir.AluOpType.mult, op1=mybir.AluOpType.mult)

                    pend_y = None
                    for m in range(6):
                        ps_qy = qky_mms(m)
                        if pend_y is not None:
                            qky_epilogue(*pend_y)
                        pend_y = (m, ps_qy)

                    # v_y
                    for ti, (o, n) in enumerate(yt_sizes):
                        gt = NTX + ti
                        ps_vy = psp.tile([128, HC * 128], F32, tag="bank")
                        if has_bqkv_y:
                            nc.tensor.matmul(
                                ps_vy[0:n, :], ones_row[:, 0:n], bvy_sb[:],
                                start=True, stop=False)
                        for kt in range(KY):
                            nc.tensor.matmul(
                                ps_vy[0:n, :], y_bf[:, kt, o:o + n], wvy_sb[:, kt, :],
                                start=(kt == 0 and not has_bqkv_y),
                                stop=(kt == KY - 1))
                        if pend_y is not None:
                            qky_epilogue(*pend_y)
                            pend_y = None
                        nc.scalar.activation(
                            v_all[0:n, gt, :], ps_vy[0:n, :],
                            mybir.ActivationFunctionType.Copy,
                            scale=rycol[0:n, ti:ti + 1])

            # =========================================================
            # Phase D: attention + projections, pipelined per query chunk
            # =========================================================
            with tc.tile_pool(name="phD", bufs=1) as pD, \
                 tc.tile_pool(name="phDs", bufs=4) as pDs, \
                 tc.tile_pool(name="phDo", bufs=2) as pDo:
                wpx_sb = pD.tile([128, HC, DX], BF16, tag="wpx")
                nc.gpsimd.dma_start(
                    wpx_sb[:], wpx_d.ap().rearrange("(h p) f -> p h f", p=128))
                if crop:
                    wpy_sb = pD.tile([128, HC, DY], BF16, tag="wpy")
                    nc.gpsimd.dma_start(
                        wpy_sb[:], wpy_d.ap().rearrange("(h p) f -> p h f", p=128))

                def emit_proj(qi, o_sb):
                    q0, qn = qc_list[qi]
                    in_x = q0 < NX
                    if in_x:
                        D, wp, out_d = DX, wpx_sb, px_d
                    else:
                        D, wp, out_d = DY, wpy_sb, py_d
                    nf_chunks = _chunks(D, 512)
                    for (p0, pn) in _chunks(qn, 128):
                        stage = pDo.tile([128, D], BF16, tag=f"stage{int(in_x)}",
                                         name="stage")
                        for gi in range(0, len(nf_chunks), 3):
                            grp = nf_chunks[gi:gi + 3]
                            ps_p = [psp.tile([128, 512], F32, tag="bank",
                                             name="ps_p") for _ in grp]
                            for h in range(HC):
                                for j, (f0, fn) in enumerate(grp):
                                    nc.tensor.matmul(
                                        ps_p[j][0:pn, 0:fn],
                                        o_sb[h][:, p0:p0 + pn],
                                        wp[:, h, f0:f0 + fn],
                                        start=(h == 0), stop=(h == HC - 1))
                            for j, (f0, fn) in enumerate(grp):
                                if (gi + j) % 2 == 0:
                                    nc.vector.tensor_copy(
                                        stage[0:pn, f0:f0 + fn], ps_p[j][0:pn, 0:fn])
                                else:
                                    nc.scalar.copy(
                                        stage[0:pn, f0:f0 + fn], ps_p[j][0:pn, 0:fn])
                        r0 = q0 + p0 - (0 if in_x else NX)
                        nc.sync.dma_start(out_d.ap()[r0:r0 + pn, :], stage[0:pn, :])

                nkt = len(kt_sizes)
                prev_proj = None   # (qi, o_sb) waiting for projection
                for qi, (q0, qn) in enumerate(qc_list):
                    o_sb = []
                    for h in range(HC):
                        o_ps = psp.tile([128, 512], F32, tag="bank")
                        d_ps = psp.tile([1, 512], F32, tag="bank")
                        # one-ahead L matmul so PE isn't blocked on ACT's exp
                        l_tiles = [None] * nkt

                        def emit_l(ki):
                            k0, kn = kt_sizes[ki]
                            l_ps = psp.tile([128, 512], F32, tag="bank",
                                            name="l_ps")
                            nc.tensor.matmul(
                                l_ps[0:kn, 0:qn], k_T[h][:, k0:k0 + kn],
                                q_T[h][:, q0:q0 + qn], start=True, stop=True)
                            l_tiles[ki] = l_ps

                        emit_l(0)
                        for ki, (k0, kn) in enumerate(kt_sizes):
                            if ki + 1 < nkt:
                                emit_l(ki + 1)
                            p_sb = pDs.tile([128, 512], BF16, tag="pT")
                            nc.scalar.activation(
                                p_sb[0:kn, 0:qn], l_tiles[ki][0:kn, 0:qn],
                                mybir.ActivationFunctionType.Exp,
                                scale=INV_SQRT_HD)
                            l_tiles[ki] = None
                            nc.tensor.matmul(
                                o_ps[:, 0:qn], v_all[0:kn, ki, h * 128:(h + 1) * 128],
                                p_sb[0:kn, 0:qn], start=(ki == 0),
                                stop=(ki == nkt - 1))
                            nc.tensor.matmul(
                                d_ps[0:1, 0:qn], ones_col[0:kn, :],
                                p_sb[0:kn, 0:qn], start=(ki == 0),
                                stop=(ki == nkt - 1))
                        rd = pDs.tile([1, 512], F32, tag="rd")
                        nc.vector.reciprocal_approx_fast(rd[0:1, 0:qn],
                                                         d_ps[0:1, 0:qn])
                        rdb = pDs.tile([128, 512], F32, tag="rdb")
                        nc.gpsimd.partition_broadcast(rdb[:, 0:qn], rd[0:1, 0:qn])
                        ot = pDs.tile([128, 512], BF16, tag=f"oT{h}",
                                      name=f"oT{h}")
                        nc.vector.tensor_tensor(
                            out=ot[:, 0:qn], in0=o_ps[:, 0:qn], in1=rdb[:, 0:qn],
                            op=mybir.AluOpType.mult)
                        o_sb.append(ot)
                        # overlap the previous chunk's projection with this
                        # chunk's remaining attention heads
                        if h == 0 and prev_proj is not None:
                            emit_proj(*prev_proj)
                            prev_proj = None
                    prev_proj = (qi, o_sb)
                emit_proj(*prev_proj)

    nc.compile()
    return nc


_CACHE = {}


def _get_nc(crop, has_bqkv_x, has_bqkv_y):
    key = (crop, has_bqkv_x, has_bqkv_y)
    if key not in _CACHE:
        _CACHE[key] = build(*key)
    return _CACHE[key]


def _install_profile_hook():
    try:
        from antenv.axon_hooks import (get_axon_ntff_profile_hook,
                                       set_axon_ntff_profile_hook)
        if get_axon_ntff_profile_hook() is None:
            from trn_agent_boot.trn_boot import _ntff_profile_via_ctypes
            set_axon_ntff_profile_hook(
                _ntff_profile_via_ctypes("/opt/axon/libaxon_pjrt.so"))
    except Exception:
        pass


# permutation putting even head-dims first, odd second (for on-chip RoPE)
_PERM = np.concatenate([np.arange(0, HD, 2), np.arange(1, HD, 2)])
# partition-swap matrix (p <-> p+64), symmetric involution
_PSW = np.zeros((HD, HD), np.float32)
_PSW[np.arange(HD), (np.arange(HD) + 64) % HD] = 1.0


def kernel(x, y, scale_x, scale_y, rope_cos, rope_sin,
           w_qkv_x, b_qkv_x, w_qkv_y, b_qkv_y,
           qnx_w, knx_w, qny_w, kny_w,
           w_proj_x, b_proj_x, w_proj_y, b_proj_y, crop_y):
    global LAST_RESULT
    crop = int(crop_y)
    f32 = np.float32
    x = np.asarray(x, f32)
    y = np.asarray(y, f32)

    has_bx = bool(np.any(np.asarray(b_qkv_x)))
    has_by = bool(np.any(np.asarray(b_qkv_y)))
    trace = bool(os.environ.get("BASS_TRACE"))
    if trace:
        _install_profile_hook()
    nc = _get_nc(crop, has_bx, has_by)

    xT = np.ascontiguousarray(x[0].T)
    sx = (1.0 + np.asarray(scale_x, f32)[0])
    sy = (1.0 + np.asarray(scale_y, f32)[0])
    wqx = np.asarray(w_qkv_x, f32).reshape(3, H, HD, DX)
    wqy = np.asarray(w_qkv_y, f32).reshape(3, H, HD, DY)
    bqx = np.asarray(b_qkv_x, f32).reshape(3, H, HD)
    bqy = np.asarray(b_qkv_y, f32).reshape(3, H, HD)
    wpx = np.asarray(w_proj_x, f32)
    wpy = np.asarray(w_proj_y, f32)
    cos = np.asarray(rope_cos, f32)
    sin = np.asarray(rope_sin, f32)
    qn = {0: np.asarray(qnx_w, f32), 1: np.asarray(knx_w, f32)}
    qny = {0: np.asarray(qny_w, f32), 1: np.asarray(kny_w, f32)}

    in_maps = []
    for c in range(N_CORES):
        hs = slice(c * HC, (c + 1) * HC)
        m = {}
        m["xT"] = xT
        # q/k weight block: [q h0..2 | k h0..2], head-dim rows permuted
        wqk = np.concatenate([wqx[t, hs][:, _PERM, :].reshape(HC * HD, DX)
                              for t in (0, 1)], axis=0)
        m["wqk_x"] = np.ascontiguousarray(wqk.T)
        m["wv_x"] = np.ascontiguousarray(wqx[2, hs].reshape(HC * HD, DX).T)
        m["sx"] = np.ascontiguousarray(sx.reshape(KX, 128).T)
        m["qn"] = np.ascontiguousarray(
            np.stack([qn[0][_PERM]] * HC + [qn[1][_PERM]] * HC, axis=1))
        # rope tables in [head, hd-partition, token] layout:
        #   cos2[p] = cos[p%64];  sin2[p<64] = -sin[p], sin2[p>=64] = +sin[p-64]
        cosh = cos[:, hs, :].transpose(1, 2, 0)        # (HC, 64, NX)
        sinh = sin[:, hs, :].transpose(1, 2, 0)
        m["cosT"] = np.ascontiguousarray(
            np.concatenate([cosh, cosh], axis=1))
        m["sinT"] = np.ascontiguousarray(
            np.concatenate([-sinh, sinh], axis=1))
        m["psw"] = _PSW
        m["bqk_x"] = np.concatenate(
            [bqx[t, hs][:, _PERM].reshape(1, HC * HD) for t in (0, 1)], axis=1)
        m["bv_x"] = bqx[2, hs].reshape(1, HC * HD)
        m["wpx"] = np.ascontiguousarray(wpx[:, c * HC * HD:(c + 1) * HC * HD].T)
        if crop:
            m["yT"] = np.ascontiguousarray(y[0, :crop].T)
            wqky = np.concatenate([wqy[t, hs][:, _PERM, :].reshape(HC * HD, DY)
                                   for t in (0, 1)], axis=0)
            m["wqk_y"] = np.ascontiguousarray(wqky.T)
            m["wv_y"] = np.ascontiguousarray(wqy[2, hs].reshape(HC * HD, DY).T)
            m["sy"] = np.ascontiguousarray(sy.reshape(KY, 128).T)
            m["qn_y"] = np.ascontiguousarray(
                np.stack([qny[0][_PERM]] * HC + [qny[1][_PERM]] * HC, axis=1))
            m["bqk_y"] = np.concatenate(
                [bqy[t, hs][:, _PERM].reshape(1, HC * HD) for t in (0, 1)], axis=1)
            m["bv_y"] = bqy[2, hs].reshape(1, HC * HD)
            m["wpy"] = np.ascontiguousarray(wpy[:, c * HC * HD:(c + 1) * HC * HD].T)
        in_maps.append({k: np.ascontiguousarray(v, f32) for k, v in m.items()})

    res = run_bass_kernel_spmd(nc, in_maps, core_ids=list(range(N_CORES)),
                               trace=trace)
    LAST_RESULT = res

    x_out = np.zeros((NX, DX), f32)
    y_out = np.zeros((NY, DY), f32)
    for c in range(N_CORES):
        x_out += res.results[c]["px"].astype(f32)
        if crop:
            y_out[:crop] += res.results[c]["py"].astype(f32)
    x_out += np.asarray(b_proj_x, f32)[None, :]
    y_out += np.asarray(b_proj_y, f32)[None, :]
    return x_out[None], y_out[None]
